# revision 3
# baseline (speedup 1.0000x reference)
"""BitLinear (ternary 2-bit packed weights) batched matmul on 8 trn2 NeuronCores.

out[b, o] = sum_i x[b, i] * w[o, i] + bias[o]
  x: (512, 4096) fp16, packed_weight: (11008, 256) int32 (16 x 2-bit codes
  per word; 0 -> 0, 1 -> +1, 2 -> -1), bias: (11008,) fp16.

Sharding: column-parallel over out_features. Each core handles 1376 rows of
packed_weight/bias, x is replicated; per-core outputs (512, 1376) are
concatenated on the host.

Per-core device kernel:
  - packed weights arrive as a u16 view (8 codes per u16 word), transposed so
    the contraction index i lives on SBUF partitions: word tile (128, 1376)
    for word-row chunk cb in 0..3; bit-position k in 0..7 yields the K-chunk
    (cb, k) holding i = 1024*cb + 8*p + k on partition p.  x is pre-permuted
    on the host with the same i-ordering, so the contraction matches.
  - DVE unpack per K-chunk: b0 = (w >> 2k) & 1, b1 = (w >> 2k+1) & 1 (two
    bitwise tensor_scalar), w = b0 - b1 as fp16 (tensor_tensor subtract).
  - TensorE: out(b_chunk m, o) accumulated over 32 K-chunks, x tile (128,128)
    stationary, unpacked w tile (128, <=512) moving, PSUM fp32.  Two passes
    over K (8 PSUM banks then 4) so the PE can consume K-chunks as the DVE
    produces them on the first pass.
  - bias added on PSUM->SBUF evacuation (bias rows replicated host-side).
"""

import numpy as np

import concourse.mybir as mybir
import concourse.tile as tile
from concourse import bacc
from concourse.alu_op_type import AluOpType
from concourse.bass_utils import run_bass_kernel_spmd

O, I, B = 11008, 4096, 512
NCORES = 8
OS = O // NCORES  # 1376 out-features per core
NKC = I // 128  # 32 K-chunks
NCB = 4  # u16 word-row chunks (I/8/128)
KPW = 8  # 2-bit codes per u16 word

# n-slices of the per-core out-feature dim (PSUM bank = 512 fp32)
N_SLICES = [(0, 512), (512, 512), (1024, 352)]
# (m_chunk, n_slice_ids) per PSUM pass: pass A uses 8 banks, pass B 4.
PASS_A = [(0, (0, 1, 2)), (1, (0, 1, 2)), (2, (0, 1))]
PASS_B = [(3, (0, 1, 2)), (2, (2,))]

TRACE = False
LAST_RESULT = None

_CACHED = None


def _build():
    nc = bacc.Bacc("TRN2", target_bir_lowering=False, debug=False,
                   num_devices=NCORES)
    f16 = mybir.dt.float16
    u16 = mybir.dt.uint16

    xT_d = nc.dram_tensor("xT", [128, NKC * B], f16, kind="ExternalInput")
    wp_d = nc.dram_tensor("wp", [128, NCB * OS], u16, kind="ExternalInput")
    bias_d = nc.dram_tensor("biasb", [128, OS], f16, kind="ExternalInput")
    out_d = nc.dram_tensor("out", [B, OS], f16, kind="ExternalOutput")

    with tile.TileContext(nc) as tc:
        with (
            tc.tile_pool(name="xp", bufs=1) as xp,
            tc.tile_pool(name="wpp", bufs=1) as wpp,
            tc.tile_pool(name="wup", bufs=1) as wup,
            tc.tile_pool(name="bp", bufs=1) as bp,
            tc.tile_pool(name="tp", bufs=2) as tp,
            tc.tile_pool(name="op", bufs=4) as op,
            tc.tile_pool(name="ps", bufs=8, space="PSUM") as ps,
        ):
            wp_sb = wpp.tile([128, NCB * OS], u16)
            for cb in range(NCB):
                s = slice(cb * OS, (cb + 1) * OS)
                nc.sync.dma_start(wp_sb[:, s], wp_d[:, s])

            x_sb = xp.tile([128, NKC * B], f16)
            for cb in range(NCB):
                s = slice(cb * KPW * B, (cb + 1) * KPW * B)
                nc.sync.dma_start(x_sb[:, s], xT_d[:, s])

            bias_sb = bp.tile([128, OS], f16)
            nc.sync.dma_start(bias_sb[:], bias_d[:])

            # ---- unpack: 32 K-chunks of (128, OS) fp16 in {-1, 0, +1}
            w_sb = wup.tile([128, NKC * OS], f16)
            for kc in range(NKC):
                cb, k = divmod(kc, KPW)
                src = wp_sb[:, cb * OS:(cb + 1) * OS]
                t0 = tp.tile([128, OS], u16, tag="t0")
                t1 = tp.tile([128, OS], u16, tag="t1")
                nc.vector.tensor_scalar(
                    t0[:], src, 2 * k, 1,
                    AluOpType.logical_shift_right, AluOpType.bitwise_and)
                nc.vector.tensor_scalar(
                    t1[:], src, 2 * k + 1, 1,
                    AluOpType.logical_shift_right, AluOpType.bitwise_and)
                nc.vector.tensor_tensor(
                    w_sb[:, kc * OS:(kc + 1) * OS], t0[:], t1[:],
                    AluOpType.subtract)

            # ---- matmuls
            out_sb = [op.tile([128, OS], f16, tag=f"out{m}", name=f"out_sb{m}")
                      for m in range(4)]

            def mm_pass(groups):
                psum = {}
                for m, ns in groups:
                    for n in ns:
                        _, nw = N_SLICES[n]
                        psum[(m, n)] = ps.tile([128, nw], mybir.dt.float32,
                                               tag="ps", name=f"ps_{m}_{n}")
                for kc in range(NKC):
                    for m, ns in groups:
                        lhsT = x_sb[:, kc * B + m * 128: kc * B + (m + 1) * 128]
                        for n in ns:
                            off, nw = N_SLICES[n]
                            rhs = w_sb[:, kc * OS + off: kc * OS + off + nw]
                            nc.tensor.matmul(
                                psum[(m, n)][:], lhsT, rhs,
                                start=(kc == 0), stop=(kc == NKC - 1))
                for m, ns in groups:
                    for n in ns:
                        off, nw = N_SLICES[n]
                        nc.vector.tensor_tensor(
                            out_sb[m][:, off:off + nw], psum[(m, n)][:],
                            bias_sb[:, off:off + nw], AluOpType.add)

            mm_pass(PASS_A)
            for m in (0, 1):
                nc.sync.dma_start(out_d[m * 128:(m + 1) * 128, :], out_sb[m][:])
            mm_pass(PASS_B)
            for m in (2, 3):
                nc.sync.dma_start(out_d[m * 128:(m + 1) * 128, :], out_sb[m][:])

    nc.compile()
    return nc


def _prep_inputs(x, packed_weight, bias):
    """Host-side re-layout (pure index shuffling, no unpacking)."""
    # x image, replicated: (128, 32*512) fp16.  K-chunk kc = 8*cb + k holds
    # i = 1024*cb + 8*p + k on partition p.
    xt = np.ascontiguousarray(x.T)  # (I, B)
    x_img = np.ascontiguousarray(
        xt.reshape(NCB, 128, KPW, B).transpose(1, 0, 2, 3).reshape(128, NKC * B)
    )

    pw_u16 = np.ascontiguousarray(packed_weight).view(np.uint16)  # (O, I/8)
    in_maps = []
    for c in range(NCORES):
        shard = pw_u16[c * OS:(c + 1) * OS]  # (OS, I/8)
        st = np.ascontiguousarray(shard.T)  # (I/8, OS) word j -> i = 8j..8j+7
        wp_img = np.ascontiguousarray(
            st.reshape(NCB, 128, OS).transpose(1, 0, 2).reshape(128, NCB * OS)
        )
        bias_img = np.ascontiguousarray(
            np.broadcast_to(bias[c * OS:(c + 1) * OS], (128, OS))
        )
        in_maps.append({"xT": x_img, "wp": wp_img, "biasb": bias_img})
    return in_maps


def kernel(x, packed_weight, bias):
    global _CACHED, LAST_RESULT
    if _CACHED is None:
        _CACHED = _build()
    nc = _CACHED
    in_maps = _prep_inputs(x, packed_weight, bias)
    res = run_bass_kernel_spmd(nc, in_maps, core_ids=list(range(NCORES)),
                               trace=TRACE)
    LAST_RESULT = res
    return np.concatenate([res.results[c]["out"] for c in range(NCORES)],
                          axis=1)


# revision 5
# speedup vs baseline: 1.0452x; 1.0452x over previous
"""BitLinear (ternary 2-bit packed weights) batched matmul on 8 trn2 NeuronCores.

out[b, o] = sum_i x[b, i] * w[o, i] + bias[o]
  x: (512, 4096) fp16, packed_weight: (11008, 256) int32 (16 x 2-bit codes
  per word; 0 -> 0, 1 -> +1, 2 -> -1), bias: (11008,) fp16.

Sharding: column-parallel over out_features. Each core handles 1376 rows of
packed_weight/bias, x is replicated; per-core outputs (512, 1376) are
concatenated on the host.

Per-core device kernel:
  - packed weights arrive as a u16 view (8 codes per u16 word), transposed so
    the contraction index i lives on SBUF partitions: word tile (128, 1376)
    for word-row chunk cb in 0..3; bit-position k in 0..7 yields the K-chunk
    (cb, k) holding i = 1024*cb + 8*p + k on partition p.  x is pre-permuted
    on the host with the same i-ordering, so the contraction matches.
  - DVE unpack per K-chunk: b0 = (w >> 2k) & 1, b1 = (w >> 2k+1) & 1 (two
    bitwise tensor_scalar), w = b0 - b1 as fp16 (tensor_tensor subtract).
  - TensorE: out(b_chunk m, o) accumulated over 32 K-chunks, x tile (128,128)
    stationary, unpacked w tile (128, <=512) moving, PSUM fp32.  Two passes
    over K (8 PSUM banks then 4) so the PE can consume K-chunks as the DVE
    produces them on the first pass.
  - bias added on PSUM->SBUF evacuation (bias rows replicated host-side).
"""

import numpy as np

import concourse.mybir as mybir
import concourse.tile as tile
from concourse import bacc
from concourse.alu_op_type import AluOpType
from concourse.bass_utils import run_bass_kernel_spmd

O, I, B = 11008, 4096, 512
NCORES = 8
OS = O // NCORES  # 1376 out-features per core
NKC = I // 128  # 32 K-chunks
NCB = 4  # u16 word-row chunks (I/8/128)
KPW = 8  # 2-bit codes per u16 word

# n-slices of the per-core out-feature dim (PSUM bank = 512 fp32)
N_SLICES = [(0, 512), (512, 512), (1024, 352)]
# (m_chunk, n_slice_ids) per PSUM pass: pass A uses 8 banks, pass B 4.
PASS_A = [(0, (0, 1, 2)), (1, (0, 1, 2)), (2, (0, 1))]
PASS_B = [(3, (0, 1, 2)), (2, (2,))]

TRACE = False
LAST_RESULT = None

_CACHED = None


def _build():
    nc = bacc.Bacc("TRN2", target_bir_lowering=False, debug=False,
                   num_devices=NCORES)
    f16 = mybir.dt.float16
    u16 = mybir.dt.uint16

    xT_d = nc.dram_tensor("xT", [128, NKC * B], f16, kind="ExternalInput")
    wp_d = nc.dram_tensor("wp", [128, NCB * OS], u16, kind="ExternalInput")
    bias_d = nc.dram_tensor("biasb", [128, OS], f16, kind="ExternalInput")
    out_d = nc.dram_tensor("out", [B, OS], f16, kind="ExternalOutput")

    with tile.TileContext(nc) as tc:
        with (
            tc.tile_pool(name="xp", bufs=1) as xp,
            tc.tile_pool(name="wpp", bufs=1) as wpp,
            tc.tile_pool(name="wup", bufs=1) as wup,
            tc.tile_pool(name="bp", bufs=1) as bp,
            tc.tile_pool(name="tp", bufs=2) as tp,
            tc.tile_pool(name="op", bufs=4) as op,
            tc.tile_pool(name="ps", bufs=8, space="PSUM") as ps,
        ):
            # Input DMAs: dispatch costs ~650ns on the issuing sequencer, so
            # spread across idle sequencers and order by first need.
            wp_sb = wpp.tile([128, NCB * OS], u16)
            nc.scalar.dma_start(wp_sb[:, 0:OS], wp_d[:, 0:OS])

            x_sb = xp.tile([128, NKC * B], f16)
            x_splits = [(0, 2), (2, 8), (8, 16), (16, 24), (24, 32)]
            for lo, hi in x_splits:
                nc.sync.dma_start(x_sb[:, lo * B:hi * B], xT_d[:, lo * B:hi * B])

            nc.scalar.dma_start(wp_sb[:, OS:NCB * OS], wp_d[:, OS:NCB * OS])

            bias_sb = bp.tile([128, OS], f16)
            nc.gpsimd.dma_start(bias_sb[:], bias_d[:])

            # ---- unpack: 32 K-chunks of (128, OS) fp16 in {-1, 0, +1}
            w_sb = wup.tile([128, NKC * OS], f16)
            for kc in range(NKC):
                cb, k = divmod(kc, KPW)
                src = wp_sb[:, cb * OS:(cb + 1) * OS]
                t0 = tp.tile([128, OS], u16, tag="t0")
                t1 = tp.tile([128, OS], u16, tag="t1")
                nc.vector.tensor_scalar(
                    t0[:], src, 2 * k, 1,
                    AluOpType.logical_shift_right, AluOpType.bitwise_and)
                nc.vector.tensor_scalar(
                    t1[:], src, 2 * k + 1, 1,
                    AluOpType.logical_shift_right, AluOpType.bitwise_and)
                nc.vector.tensor_tensor(
                    w_sb[:, kc * OS:(kc + 1) * OS], t0[:], t1[:],
                    AluOpType.subtract)

            # ---- matmuls
            out_sb = [op.tile([128, OS], f16, tag=f"out{m}", name=f"out_sb{m}")
                      for m in range(4)]

            def mm_pass(groups, dma_engines):
                psum = {}
                for m, ns in groups:
                    for n in ns:
                        _, nw = N_SLICES[n]
                        psum[(m, n)] = ps.tile([128, nw], mybir.dt.float32,
                                               tag="ps", name=f"ps_{m}_{n}")
                for kc in range(NKC):
                    for m, ns in groups:
                        lhsT = x_sb[:, kc * B + m * 128: kc * B + (m + 1) * 128]
                        for n in ns:
                            off, nw = N_SLICES[n]
                            rhs = w_sb[:, kc * OS + off: kc * OS + off + nw]
                            nc.tensor.matmul(
                                psum[(m, n)][:], lhsT, rhs,
                                start=(kc == 0), stop=(kc == NKC - 1))
                # evacuate + store each (m, n) slice independently so output
                # DMAs overlap the remaining evacuations
                for i, (m, n) in enumerate((m, n) for m, ns in groups
                                           for n in ns):
                    off, nw = N_SLICES[n]
                    nc.vector.tensor_tensor(
                        out_sb[m][:, off:off + nw], psum[(m, n)][:],
                        bias_sb[:, off:off + nw], AluOpType.add)
                    eng = dma_engines[i % len(dma_engines)]
                    eng.dma_start(
                        out_d[m * 128:(m + 1) * 128, off:off + nw],
                        out_sb[m][:, off:off + nw])

            mm_pass(PASS_A, [nc.gpsimd, nc.scalar])
            mm_pass(PASS_B, [nc.gpsimd, nc.scalar])

    nc.compile()
    return nc


def _prep_inputs(x, packed_weight, bias):
    """Host-side re-layout (pure index shuffling, no unpacking)."""
    # x image, replicated: (128, 32*512) fp16.  K-chunk kc = 8*cb + k holds
    # i = 1024*cb + 8*p + k on partition p.
    xt = np.ascontiguousarray(x.T)  # (I, B)
    x_img = np.ascontiguousarray(
        xt.reshape(NCB, 128, KPW, B).transpose(1, 0, 2, 3).reshape(128, NKC * B)
    )

    pw_u16 = np.ascontiguousarray(packed_weight).view(np.uint16)  # (O, I/8)
    in_maps = []
    for c in range(NCORES):
        shard = pw_u16[c * OS:(c + 1) * OS]  # (OS, I/8)
        st = np.ascontiguousarray(shard.T)  # (I/8, OS) word j -> i = 8j..8j+7
        wp_img = np.ascontiguousarray(
            st.reshape(NCB, 128, OS).transpose(1, 0, 2).reshape(128, NCB * OS)
        )
        bias_img = np.ascontiguousarray(
            np.broadcast_to(bias[c * OS:(c + 1) * OS], (128, OS))
        )
        in_maps.append({"xT": x_img, "wp": wp_img, "biasb": bias_img})
    return in_maps


def kernel(x, packed_weight, bias):
    global _CACHED, LAST_RESULT
    if _CACHED is None:
        _CACHED = _build()
    nc = _CACHED
    in_maps = _prep_inputs(x, packed_weight, bias)
    res = run_bass_kernel_spmd(nc, in_maps, core_ids=list(range(NCORES)),
                               trace=TRACE)
    LAST_RESULT = res
    return np.concatenate([res.results[c]["out"] for c in range(NCORES)],
                          axis=1)
